# revision 7
# baseline (speedup 1.0000x reference)
"""Trainium2 Bass kernel for nn_Critic_QuadNeighborhood_MultiheadAttention.

Data-parallel over 8 NeuronCores: batch (16384) is sharded into 8
contiguous chunks of 2048 rows (= 256 groups of 8 agents, group
boundaries preserved).  All weights are replicated.

Layout on device is feature-major ("transposed"): activations live as
[feature_partitions, rows] so every MLP layer is a plain
weights-stationary matmul chain with no transposes between layers.  The
two softmax-attention stages use DVE grouped reductions along the free
axis plus PE ones-matmuls for partition-axis sums (which also broadcast
scores across partitions for free).  Final outputs are transposed back
to row-major with PE transpose ops.
"""

import numpy as np

import concourse.bass as bass
import concourse.tile as tile
from concourse import bacc, mybir
from concourse.bass import ds, ts
from concourse.bass_utils import run_bass_kernel_spmd

F32 = mybir.dt.float32
F32R = mybir.dt.float32r
AF = mybir.ActivationFunctionType
AX = mybir.AxisListType
OP = mybir.AluOpType

N_CORES = 8
B = 16384
SD = 18          # self obs dim
ND = 6           # neighbor obs dim
NN = 7           # num neighbors
H = 512
NA = 8           # agents per group
BC = B // N_CORES          # 2048 batch rows per core
R = BC * NN                # 14336 neighbor rows per core
CW = 448                   # chunk width (cols) = 64 batches * 7
NCH = R // CW              # 32 chunks
CB = CW // NN              # 64 batches per chunk
GC = BC // NA              # 256 groups per core
MT = H // 128              # 4 feature tiles of 128
C1 = 1.0 / (NN * float(np.sqrt(H)))        # stage-1 score scale (mean + 1/sqrt(H))
C2 = 1.0 / (NA * float(np.sqrt(H // NA)))  # stage-2 score scale = 1/64

# matmul input dtype: float32 exact (4 cyc/row) or float32r (1 cyc/row)
USE_F32R = False

_CACHE = {}


def _mm_ap(ap):
    return ap.bitcast(F32R) if USE_F32R else ap


def _build_program():
    nc = bacc.Bacc("TRN2", target_bir_lowering=False, debug=False,
                   num_devices=N_CORES)

    din = {}
    for name, shape in [
        ("xt", [SD + ND, R]),
        ("we1", [SD + ND, H]),
        ("we2", [128, MT * H]),
        ("wn1", [128, MT * H]),
        ("wn2", [128, MT * H]),
        ("wa1", [128, MT * H]),
        ("wa2", [128, MT * H]),
        ("be1", [128, MT]),
        ("be2", [128, MT]),
        ("bn1", [128, MT]),
        ("bn2", [128, MT]),
        ("ba1", [128, MT]),
        ("ba2", [128, MT]),
        ("ones_sc", [128, 128]),
        ("head_ones", [128, 128]),
        ("ident", [128, 128]),
    ]:
        din[name] = nc.dram_tensor(name, shape, F32, kind="ExternalInput").ap()
    a_out = nc.dram_tensor("a_out", [BC, H], F32, kind="ExternalOutput").ap()
    mh_out = nc.dram_tensor("mh_out", [GC, H], F32, kind="ExternalOutput").ap()

    with tile.TileContext(nc) as tc:
        _body(nc, tc, din, a_out, mh_out)
    nc.compile()
    return nc


def _body(nc, tc, din, a_out, mh_out):
    from contextlib import ExitStack
    ctx = ExitStack()
    with ctx:
        cpool = ctx.enter_context(tc.tile_pool(name="consts", bufs=1))
        wk = ctx.enter_context(tc.tile_pool(name="wk", bufs=1))
        pp = ctx.enter_context(tc.tile_pool(name="pp", bufs=1, space="PSUM"))

        # ---- load constants ----
        def cload(name, shape):
            t = cpool.tile(shape, F32, name=f"c_{name}", tag=f"c_{name}")
            nc.sync.dma_start(t[:], din[name][:])
            return t

        we1 = cload("we1", [SD + ND, H])
        W = {k: cload(k, [128, MT * H]) for k in ("we2", "wn1", "wn2", "wa1", "wa2")}
        BV = {k: cload(k, [128, MT]) for k in ("be1", "be2", "bn1", "bn2", "ba1", "ba2")}
        ones_sc = cload("ones_sc", [128, 128])
        head_ones = cload("head_ones", [128, 128])
        ident = cload("ident", [128, 128])

        # ---- persistent activations ----
        AT = [cpool.tile([128, BC], F32, name=f"at{m}", tag=f"at{m}")
              for m in range(MT)]
        MTT = [cpool.tile([128, GC], F32, name=f"mt{m}", tag=f"mt{m}")
               for m in range(MT)]

        def mlp_layer(src, wname, bname, tagf, n_cols):
            """One tanh(W.T @ src + b) layer in feature-major layout.

            src: list of MT APs [128, n_cols] (or a single [K, n_cols] AP
            for the first layer).  Returns list of MT APs [128, n_cols].
            """
            out = []
            for m in range(MT):
                ps = pp.tile([128, n_cols], F32, tag="mme", bufs=4,
                             name=f"ps_{wname}_{m}")
                if wname == "we1":
                    nc.tensor.matmul(ps[:], _mm_ap(we1[:, ts(m, 128)]),
                                     _mm_ap(src), start=True, stop=True)
                else:
                    for k in range(MT):
                        nc.tensor.matmul(
                            ps[:],
                            _mm_ap(W[wname][:, ds(k * H + m * 128, 128)]),
                            _mm_ap(src[k]),
                            start=(k == 0), stop=(k == MT - 1))
                t = wk.tile([128, n_cols], F32, tag=tagf.format(m=m), bufs=2,
                            name=f"t_{wname}_{m}")
                nc.scalar.activation(t[:], ps[:], AF.Tanh,
                                     bias=BV[bname][:, m:m + 1])
                out.append(t[:])
            return out

        # ================= stage C: per-chunk fused pipeline =================
        for ch in range(NCH):
            cs = ds(ch * CW, CW)
            xt = wk.tile([SD + ND, CW], F32, tag="xt", bufs=2, name="xt")
            nc.sync.dma_start(xt[:], din["xt"][:, cs])

            h1 = mlp_layer(xt[:], "we1", "be1", "bigA_{m}", CW)
            E = mlp_layer(h1, "we2", "be2", "bigB_{m}", CW)

            # scores = ones_sc.T @ (E * Msum_bcast)  (broadcast over all parts)
            ps_s = pp.tile([128, CW], F32, tag="msc", bufs=2, name="ps_s")
            for m in range(MT):
                msum = wk.tile([128, CB], F32, tag=f"sm_{m}", bufs=2,
                               name=f"msum{m}")
                nc.vector.tensor_reduce(
                    msum[:], E[m].rearrange("p (b n) -> p b n", n=NN),
                    axis=AX.X, op=OP.add)
                pm = wk.tile([128, CW], F32, tag=f"bigF_{m}", bufs=2,
                             name=f"pm{m}")
                nc.gpsimd.tensor_mul(
                    pm[:].rearrange("p (b n) -> p b n", n=NN),
                    E[m].rearrange("p (b n) -> p b n", n=NN),
                    msum[:].rearrange("p b -> p b ()").broadcast_to([128, CB, NN]))
                nc.tensor.matmul(ps_s[:], _mm_ap(ones_sc[:]), _mm_ap(pm[:]),
                                 start=(m == 0), stop=(m == MT - 1))

            T1 = mlp_layer(E, "wn1", "bn1", "bigC_{m}", CW)
            V = mlp_layer(T1, "wn2", "bn2", "bigD_{m}", CW)

            eS = wk.tile([128, CW], F32, tag="eS", bufs=2, name="eS")
            nc.scalar.activation(eS[:], ps_s[:], AF.Exp)
            den = wk.tile([128, CB], F32, tag="den", bufs=2, name="den")
            nc.vector.tensor_reduce(
                den[:], eS[:].rearrange("p (b n) -> p b n", n=NN),
                axis=AX.X, op=OP.add)
            rden = wk.tile([128, CB], F32, tag="rden", bufs=2, name="rden")
            nc.vector.reciprocal(rden[:], den[:])
            prob = wk.tile([128, CW], F32, tag="prob", bufs=2, name="prob")
            nc.vector.tensor_mul(
                prob[:].rearrange("p (b n) -> p b n", n=NN),
                eS[:].rearrange("p (b n) -> p b n", n=NN),
                rden[:].rearrange("p b -> p b ()").broadcast_to([128, CB, NN]))

            for m in range(MT):
                w8 = wk.tile([128, CW], F32, tag=f"bigE_{m}", bufs=2,
                             name=f"w8{m}")
                nc.vector.tensor_mul(w8[:], V[m], prob[:])
                nc.vector.tensor_reduce(
                    AT[m][:, ds(ch * CB, CB)],
                    w8[:].rearrange("p (b n) -> p b n", n=NN),
                    axis=AX.X, op=OP.add)

        # ================= stage D: cross-agent multihead attention ==========
        for n in range(MT):
            nsl = ds(n * 512, 512)
            Ta = mlp_layer([AT[k][:, nsl] for k in range(MT)],
                           "wa1", "ba1", "bigA_{m}", 512)
            AV = mlp_layer(Ta, "wa2", "ba2", "bigB_{m}", 512)

            for m in range(MT):
                gm = wk.tile([128, 64], F32, tag=f"sm_{m}", bufs=2,
                             name=f"gm{m}")
                nc.vector.tensor_reduce(
                    gm[:], AT[m][:, nsl].rearrange("p (g a) -> p g a", a=NA),
                    axis=AX.X, op=OP.add)
                pg = wk.tile([128, 512], F32, tag=f"bigC_{m}", bufs=2,
                             name=f"pg{m}")
                nc.gpsimd.tensor_mul(
                    pg[:].rearrange("p (g a) -> p g a", a=NA),
                    AT[m][:, nsl].rearrange("p (g a) -> p g a", a=NA),
                    gm[:].rearrange("p g -> p g ()").broadcast_to([128, 64, NA]))
                ps_h = pp.tile([128, 512], F32, tag="msc", bufs=2,
                               name=f"psh{m}")
                nc.tensor.matmul(ps_h[:], _mm_ap(head_ones[:]), _mm_ap(pg[:]),
                                 start=True, stop=True)
                eh = wk.tile([128, 512], F32, tag=f"bigD_{m}", bufs=2,
                             name=f"eh{m}")
                nc.scalar.activation(eh[:], ps_h[:], AF.Exp)
                dh = wk.tile([128, 64], F32, tag=f"dh_{m}", bufs=2,
                             name=f"dh{m}")
                nc.vector.tensor_reduce(
                    dh[:], eh[:].rearrange("p (g a) -> p g a", a=NA),
                    axis=AX.X, op=OP.add)
                rh = wk.tile([128, 64], F32, tag=f"rh_{m}", bufs=2,
                             name=f"rh{m}")
                nc.vector.reciprocal(rh[:], dh[:])
                ph = wk.tile([128, 512], F32, tag=f"bigF_{m}", bufs=2,
                             name=f"ph{m}")
                nc.vector.tensor_mul(
                    ph[:].rearrange("p (g a) -> p g a", a=NA),
                    eh[:].rearrange("p (g a) -> p g a", a=NA),
                    rh[:].rearrange("p g -> p g ()").broadcast_to([128, 64, NA]))
                wg = wk.tile([128, 512], F32, tag=f"bigE_{m}", bufs=2,
                             name=f"wg{m}")
                nc.vector.tensor_mul(wg[:], AV[m], ph[:])
                nc.vector.tensor_reduce(
                    MTT[m][:, ds(n * 64, 64)],
                    wg[:].rearrange("p (g a) -> p g a", a=NA),
                    axis=AX.X, op=OP.add)

        # ================= stage E: transpose + store outputs ================
        for c in range(GC // 128):          # 2 tiles of 128 group-rows
            mh_sb = wk.tile([128, 512], F32, tag="osb", bufs=2, name="mh_sb")
            for m in range(MT):
                pt = pp.tile([128, 128], F32, tag="mme", bufs=4, name="pt")
                nc.tensor.transpose(pt[:], MTT[m][:, ts(c, 128)], ident[:])
                nc.scalar.copy(mh_sb[:, ts(m, 128)], pt[:])
            nc.sync.dma_start(mh_out[ts(c, 128), :], mh_sb[:])

        for c in range(BC // 128):          # 16 tiles of 128 batch-rows
            a_sb = wk.tile([128, 512], F32, tag="osb", bufs=2, name="a_sb")
            for m in range(MT):
                pt = pp.tile([128, 128], F32, tag="mme", bufs=4, name="pt")
                nc.tensor.transpose(pt[:], AT[m][:, ts(c, 128)], ident[:])
                nc.scalar.copy(a_sb[:, ts(m, 128)], pt[:])
            nc.sync.dma_start(a_out[ts(c, 128), :], a_sb[:])


def _get_program():
    if "nc" not in _CACHE:
        _CACHE["nc"] = _build_program()
    return _CACHE["nc"]


def _prep_inputs(self_obs, obs, W_e1, b_e1, W_e2, b_e2, W_n1, b_n1,
                 W_n2, b_n2, W_a1, b_a1, W_a2, b_a2):
    """Build the per-core input maps (host-side shard + transpose)."""
    self_obs = np.asarray(self_obs, np.float32)
    obs = np.asarray(obs, np.float32)
    obs_nb = np.ascontiguousarray(obs[:, SD:SD + ND * NN]).reshape(-1, ND)

    def wsb(Wm):
        Wm = np.asarray(Wm, np.float32)
        return np.ascontiguousarray(
            Wm.reshape(MT, 128, H).transpose(1, 0, 2).reshape(128, MT * H))

    def bsb(b):
        return np.ascontiguousarray(np.asarray(b, np.float32).reshape(MT, 128).T)

    shared = {
        "we1": np.asarray(W_e1, np.float32),
        "we2": wsb(W_e2), "wn1": wsb(W_n1), "wn2": wsb(W_n2),
        "wa1": wsb(W_a1), "wa2": wsb(W_a2),
        "be1": bsb(b_e1), "be2": bsb(b_e2), "bn1": bsb(b_n1),
        "bn2": bsb(b_n2), "ba1": bsb(b_a1), "ba2": bsb(b_a2),
        "ones_sc": np.full((128, 128), C1, np.float32),
        "head_ones": np.kron(np.eye(2, dtype=np.float32),
                             np.ones((64, 64), np.float32)) * np.float32(C2),
        "ident": np.eye(128, dtype=np.float32),
    }

    in_maps = []
    for c in range(N_CORES):
        i0 = c * R
        idx = (i0 + np.arange(R)) % B
        xc = np.concatenate([self_obs[idx], obs_nb[i0:i0 + R]], axis=1)  # [R, 24]
        xt = np.ascontiguousarray(xc.T)                                  # [24, R]
        in_maps.append({"xt": xt, **shared})
    return in_maps


def _run(in_maps, trace=False):
    nc = _get_program()
    return run_bass_kernel_spmd(nc, in_maps, core_ids=list(range(N_CORES)),
                                trace=trace)


def kernel(self_obs, obs, W_e1, b_e1, W_e2, b_e2, W_n1, b_n1, W_n2, b_n2,
           W_a1, b_a1, W_a2, b_a2, all_neighbor_obs_size=None,
           batch_size=None, num_groups=None, **_unused):
    in_maps = _prep_inputs(self_obs, obs, W_e1, b_e1, W_e2, b_e2, W_n1, b_n1,
                           W_n2, b_n2, W_a1, b_a1, W_a2, b_a2)
    res = _run(in_maps)
    agent_attention = np.concatenate(
        [res.results[c]["a_out"] for c in range(N_CORES)], axis=0)
    multi = np.concatenate(
        [res.results[c]["mh_out"] for c in range(N_CORES)], axis=0)
    multi_head_attention = np.tile(multi, (NA, 1))
    return (multi_head_attention, agent_attention)


# revision 11
# speedup vs baseline: 2.7566x; 2.7566x over previous
"""Trainium2 Bass kernel for nn_Critic_QuadNeighborhood_MultiheadAttention.

Data-parallel over 8 NeuronCores: batch (16384) is sharded into 8
contiguous chunks of 2048 rows (= 256 groups of 8 agents, group
boundaries preserved).  All weights are replicated.

Layout on device is feature-major ("transposed"): activations live as
[feature_partitions, rows] so every MLP layer is a plain
weights-stationary matmul chain with no transposes between layers.  The
two softmax-attention stages use DVE grouped reductions along the free
axis plus PE ones-matmuls for partition-axis sums (which also broadcast
scores across partitions for free).  Final outputs are transposed back
to row-major with PE transpose ops.

Matmuls run in float32r (1 cycle/row on the PE vs 4 for fp32, ~2e-4
matmul rel err).  The BIR verifier requires every float32r matmul
operand to be *produced* as float32r, so matmul-feeding tiles are
declared float32r at their producer (DMA / ACT / DVE / GPSIMD) and
bitcast back to fp32 wherever a vector engine reads them.
"""

import numpy as np

import concourse.bass as bass
import concourse.tile as tile
from concourse import bacc, mybir
from concourse.bass import ds, ts
from concourse.bass_utils import run_bass_kernel_spmd

F32 = mybir.dt.float32
F32R = mybir.dt.float32r
AF = mybir.ActivationFunctionType
AX = mybir.AxisListType
OP = mybir.AluOpType

N_CORES = 8
B = 16384
SD = 18          # self obs dim
ND = 6           # neighbor obs dim
NN = 7           # num neighbors
H = 512
NA = 8           # agents per group
BC = B // N_CORES          # 2048 batch rows per core
R = BC * NN                # 14336 neighbor rows per core
CW = 448                   # chunk width (cols) = 64 batches * 7
NCH = R // CW              # 32 chunks
CB = CW // NN              # 64 batches per chunk
GC = BC // NA              # 256 groups per core
MT = H // 128              # 4 feature tiles of 128
C1 = 1.0 / (NN * float(np.sqrt(H)))        # stage-1 score scale (mean + 1/sqrt(H))
C2 = 1.0 / (NA * float(np.sqrt(H // NA)))  # stage-2 score scale = 1/64

# matmul input dtype: float32 exact (4 cyc/row) or float32r (1 cyc/row)
USE_F32R = True
MMDT = F32R if USE_F32R else F32

_CACHE = {}


def _f32(ap):
    """View a (possibly float32r) AP as plain fp32 for vector-engine use."""
    return ap.bitcast(F32) if USE_F32R else ap


def _build_program():
    nc = bacc.Bacc("TRN2", target_bir_lowering=False, debug=False,
                   num_devices=N_CORES)

    din = {}
    f32r_inputs = {"xt", "we1", "we2", "wn1", "wn2", "wa1", "wa2",
                   "ones_sc", "head_ones"}
    for name, shape in [
        ("xt", [SD + ND, R]),
        ("we1", [SD + ND, H]),
        ("we2", [128, MT * H]),
        ("wn1", [128, MT * H]),
        ("wn2", [128, MT * H]),
        ("wa1", [128, MT * H]),
        ("wa2", [128, MT * H]),
        ("be1", [128, MT]),
        ("be2", [128, MT]),
        ("bn1", [128, MT]),
        ("bn2", [128, MT]),
        ("ba1", [128, MT]),
        ("ba2", [128, MT]),
        ("ones_sc", [128, 128]),
        ("head_ones", [128, 128]),
        ("ident", [128, 128]),
    ]:
        dt = MMDT if name in f32r_inputs else F32
        din[name] = nc.dram_tensor(name, shape, dt, kind="ExternalInput").ap()
    a_out = nc.dram_tensor("a_out", [BC, H], F32, kind="ExternalOutput").ap()
    mh_out = nc.dram_tensor("mh_out", [GC, H], F32, kind="ExternalOutput").ap()

    with tile.TileContext(nc) as tc:
        _body(nc, tc, din, a_out, mh_out)
    nc.compile()
    return nc


def _body(nc, tc, din, a_out, mh_out):
    from contextlib import ExitStack
    ctx = ExitStack()
    with ctx:
        cpool = ctx.enter_context(tc.tile_pool(name="consts", bufs=1))
        wk = ctx.enter_context(tc.tile_pool(name="wk", bufs=1))
        pp = ctx.enter_context(tc.tile_pool(name="pp", bufs=1, space="PSUM"))

        # ---- load constants ----
        def cload(name, shape):
            t = cpool.tile(shape, din[name].dtype, name=f"c_{name}",
                           tag=f"c_{name}")
            nc.sync.dma_start(t[:], din[name][:])
            return t

        we1 = cload("we1", [SD + ND, H])
        W = {k: cload(k, [128, MT * H]) for k in ("we2", "wn1", "wn2", "wa1", "wa2")}
        BV = {k: cload(k, [128, MT]) for k in ("be1", "be2", "bn1", "bn2", "ba1", "ba2")}
        ones_sc = cload("ones_sc", [128, 128])
        head_ones = cload("head_ones", [128, 128])
        ident = cload("ident", [128, 128])

        # ---- persistent activations (kept fp32: these become outputs) ----
        AT = [cpool.tile([128, BC], F32, name=f"at{m}", tag=f"at{m}")
              for m in range(MT)]
        MTT = [cpool.tile([128, GC], F32, name=f"mt{m}", tag=f"mt{m}")
               for m in range(MT)]

        def mlp_layer(src, wname, bname, tagf, n_cols, out_dt):
            """One tanh(W.T @ src + b) layer in feature-major layout.

            src: list of MT APs [128, n_cols] (or a single [K, n_cols] AP
            for the first layer), dtype MMDT.  Returns list of MT APs.
            """
            out = []
            for m in range(MT):
                ps = pp.tile([128, n_cols], F32, tag="mme", bufs=4,
                             name=f"ps_{wname}_{m}")
                if wname == "we1":
                    nc.tensor.matmul(ps[:], we1[:, ts(m, 128)], src,
                                     start=True, stop=True)
                else:
                    for k in range(MT):
                        nc.tensor.matmul(
                            ps[:],
                            W[wname][:, ds(k * H + m * 128, 128)],
                            src[k],
                            start=(k == 0), stop=(k == MT - 1))
                t = wk.tile([128, n_cols], out_dt, tag=tagf.format(m=m), bufs=2,
                            name=f"t_{wname}_{m}")
                nc.scalar.activation(t[:], ps[:], AF.Tanh,
                                     bias=BV[bname][:, m:m + 1])
                out.append(t[:])
            return out

        # ================= stage C: per-chunk fused pipeline =================
        for ch in range(NCH):
            cs = ds(ch * CW, CW)
            xt = wk.tile([SD + ND, CW], MMDT, tag="xt", bufs=2, name="xt")
            nc.sync.dma_start(xt[:], din["xt"][:, cs])

            h1 = mlp_layer(xt[:], "we1", "be1", "bigA_{m}", CW, MMDT)
            E = mlp_layer(h1, "we2", "be2", "bigB_{m}", CW, MMDT)

            # scores = ones_sc.T @ (E * Msum_bcast)  (broadcast over all parts)
            ps_s = pp.tile([128, CW], F32, tag="msc", bufs=2, name="ps_s")
            for m in range(MT):
                msum = wk.tile([128, CB], F32, tag=f"sm_{m}", bufs=2,
                               name=f"msum{m}")
                nc.vector.tensor_reduce(
                    msum[:], _f32(E[m]).rearrange("p (b n) -> p b n", n=NN),
                    axis=AX.X, op=OP.add)
                pm = wk.tile([128, CW], MMDT, tag=f"bigF_{m}", bufs=2,
                             name=f"pm{m}")
                nc.gpsimd.tensor_mul(
                    pm[:].rearrange("p (b n) -> p b n", n=NN),
                    _f32(E[m]).rearrange("p (b n) -> p b n", n=NN),
                    msum[:].rearrange("p b -> p b ()").broadcast_to([128, CB, NN]))
                nc.tensor.matmul(ps_s[:], ones_sc[:], pm[:],
                                 start=(m == 0), stop=(m == MT - 1))

            T1 = mlp_layer(E, "wn1", "bn1", "bigC_{m}", CW, MMDT)
            V = mlp_layer(T1, "wn2", "bn2", "bigD_{m}", CW, F32)

            eS = wk.tile([128, CW], F32, tag="eS", bufs=2, name="eS")
            nc.scalar.activation(eS[:], ps_s[:], AF.Exp)
            den = wk.tile([128, CB], F32, tag="den", bufs=2, name="den")
            nc.vector.tensor_reduce(
                den[:], eS[:].rearrange("p (b n) -> p b n", n=NN),
                axis=AX.X, op=OP.add)
            rden = wk.tile([128, CB], F32, tag="rden", bufs=2, name="rden")
            nc.vector.reciprocal(rden[:], den[:])
            prob = wk.tile([128, CW], F32, tag="prob", bufs=2, name="prob")
            nc.vector.tensor_mul(
                prob[:].rearrange("p (b n) -> p b n", n=NN),
                eS[:].rearrange("p (b n) -> p b n", n=NN),
                rden[:].rearrange("p b -> p b ()").broadcast_to([128, CB, NN]))

            for m in range(MT):
                w8 = wk.tile([128, CW], F32, tag=f"bigE_{m}", bufs=2,
                             name=f"w8{m}")
                nc.vector.tensor_mul(w8[:], V[m], prob[:])
                nc.vector.tensor_reduce(
                    AT[m][:, ds(ch * CB, CB)],
                    w8[:].rearrange("p (b n) -> p b n", n=NN),
                    axis=AX.X, op=OP.add)

        # ================= stage D: cross-agent multihead attention ==========
        for n in range(MT):
            nsl = ds(n * 512, 512)
            # float32r copies of A^T for the a1 matmuls
            ATr = []
            for k in range(MT):
                atr = wk.tile([128, 512], MMDT, tag=f"atr_{k}", bufs=1,
                              name=f"atr{k}")
                nc.vector.tensor_copy(atr[:], AT[k][:, nsl])
                ATr.append(atr[:])
            Ta = mlp_layer(ATr, "wa1", "ba1", "bigA_{m}", 512, MMDT)
            AV = mlp_layer(Ta, "wa2", "ba2", "bigB_{m}", 512, F32)

            for m in range(MT):
                gm = wk.tile([128, 64], F32, tag=f"sm_{m}", bufs=2,
                             name=f"gm{m}")
                nc.vector.tensor_reduce(
                    gm[:], AT[m][:, nsl].rearrange("p (g a) -> p g a", a=NA),
                    axis=AX.X, op=OP.add)
                pg = wk.tile([128, 512], MMDT, tag=f"bigC_{m}", bufs=2,
                             name=f"pg{m}")
                nc.gpsimd.tensor_mul(
                    pg[:].rearrange("p (g a) -> p g a", a=NA),
                    AT[m][:, nsl].rearrange("p (g a) -> p g a", a=NA),
                    gm[:].rearrange("p g -> p g ()").broadcast_to([128, 64, NA]))
                ps_h = pp.tile([128, 512], F32, tag="msc", bufs=2,
                               name=f"psh{m}")
                nc.tensor.matmul(ps_h[:], head_ones[:], pg[:],
                                 start=True, stop=True)
                eh = wk.tile([128, 512], F32, tag=f"bigD_{m}", bufs=2,
                             name=f"eh{m}")
                nc.scalar.activation(eh[:], ps_h[:], AF.Exp)
                dh = wk.tile([128, 64], F32, tag=f"dh_{m}", bufs=2,
                             name=f"dh{m}")
                nc.vector.tensor_reduce(
                    dh[:], eh[:].rearrange("p (g a) -> p g a", a=NA),
                    axis=AX.X, op=OP.add)
                rh = wk.tile([128, 64], F32, tag=f"rh_{m}", bufs=2,
                             name=f"rh{m}")
                nc.vector.reciprocal(rh[:], dh[:])
                ph = wk.tile([128, 512], F32, tag=f"bigF_{m}", bufs=2,
                             name=f"ph{m}")
                nc.vector.tensor_mul(
                    ph[:].rearrange("p (g a) -> p g a", a=NA),
                    eh[:].rearrange("p (g a) -> p g a", a=NA),
                    rh[:].rearrange("p g -> p g ()").broadcast_to([128, 64, NA]))
                wg = wk.tile([128, 512], F32, tag=f"bigE_{m}", bufs=2,
                             name=f"wg{m}")
                nc.vector.tensor_mul(wg[:], AV[m], ph[:])
                nc.vector.tensor_reduce(
                    MTT[m][:, ds(n * 64, 64)],
                    wg[:].rearrange("p (g a) -> p g a", a=NA),
                    axis=AX.X, op=OP.add)

        # ================= stage E: transpose + store outputs ================
        for c in range(GC // 128):          # 2 tiles of 128 group-rows
            mh_sb = wk.tile([128, 512], F32, tag="osb", bufs=2, name="mh_sb")
            for m in range(MT):
                pt = pp.tile([128, 128], F32, tag="mme", bufs=4, name="pt")
                nc.tensor.transpose(pt[:], MTT[m][:, ts(c, 128)], ident[:])
                nc.scalar.copy(mh_sb[:, ts(m, 128)], pt[:])
            nc.sync.dma_start(mh_out[ts(c, 128), :], mh_sb[:])

        for c in range(BC // 128):          # 16 tiles of 128 batch-rows
            a_sb = wk.tile([128, 512], F32, tag="osb", bufs=2, name="a_sb")
            for m in range(MT):
                pt = pp.tile([128, 128], F32, tag="mme", bufs=4, name="pt")
                nc.tensor.transpose(pt[:], AT[m][:, ts(c, 128)], ident[:])
                nc.scalar.copy(a_sb[:, ts(m, 128)], pt[:])
            nc.sync.dma_start(a_out[ts(c, 128), :], a_sb[:])


def _get_program():
    if "nc" not in _CACHE:
        _CACHE["nc"] = _build_program()
    return _CACHE["nc"]


def _prep_inputs(self_obs, obs, W_e1, b_e1, W_e2, b_e2, W_n1, b_n1,
                 W_n2, b_n2, W_a1, b_a1, W_a2, b_a2):
    """Build the per-core input maps (host-side shard + transpose)."""
    self_obs = np.asarray(self_obs, np.float32)
    obs = np.asarray(obs, np.float32)
    obs_nb = np.ascontiguousarray(obs[:, SD:SD + ND * NN]).reshape(-1, ND)

    def wsb(Wm):
        Wm = np.asarray(Wm, np.float32)
        return np.ascontiguousarray(
            Wm.reshape(MT, 128, H).transpose(1, 0, 2).reshape(128, MT * H))

    def bsb(b):
        return np.ascontiguousarray(np.asarray(b, np.float32).reshape(MT, 128).T)

    shared = {
        "we1": np.asarray(W_e1, np.float32),
        "we2": wsb(W_e2), "wn1": wsb(W_n1), "wn2": wsb(W_n2),
        "wa1": wsb(W_a1), "wa2": wsb(W_a2),
        "be1": bsb(b_e1), "be2": bsb(b_e2), "bn1": bsb(b_n1),
        "bn2": bsb(b_n2), "ba1": bsb(b_a1), "ba2": bsb(b_a2),
        "ones_sc": np.full((128, 128), C1, np.float32),
        "head_ones": np.kron(np.eye(2, dtype=np.float32),
                             np.ones((64, 64), np.float32)) * np.float32(C2),
        "ident": np.eye(128, dtype=np.float32),
    }

    in_maps = []
    for c in range(N_CORES):
        i0 = c * R
        idx = (i0 + np.arange(R)) % B
        xc = np.concatenate([self_obs[idx], obs_nb[i0:i0 + R]], axis=1)  # [R, 24]
        xt = np.ascontiguousarray(xc.T)                                  # [24, R]
        in_maps.append({"xt": xt, **shared})
    return in_maps


def _run(in_maps, trace=False):
    nc = _get_program()
    return run_bass_kernel_spmd(nc, in_maps, core_ids=list(range(N_CORES)),
                                trace=trace)


def kernel(self_obs, obs, W_e1, b_e1, W_e2, b_e2, W_n1, b_n1, W_n2, b_n2,
           W_a1, b_a1, W_a2, b_a2, all_neighbor_obs_size=None,
           batch_size=None, num_groups=None, **_unused):
    in_maps = _prep_inputs(self_obs, obs, W_e1, b_e1, W_e2, b_e2, W_n1, b_n1,
                           W_n2, b_n2, W_a1, b_a1, W_a2, b_a2)
    res = _run(in_maps)
    agent_attention = np.concatenate(
        [res.results[c]["a_out"] for c in range(N_CORES)], axis=0)
    multi = np.concatenate(
        [res.results[c]["mh_out"] for c in range(N_CORES)], axis=0)
    multi_head_attention = np.tile(multi, (NA, 1))
    return (multi_head_attention, agent_attention)
